# revision 15
# baseline (speedup 1.0000x reference)
"""Expert-parallel MoE MLP kernel for TRN2 (8 NeuronCores).

Reference computation (all experts, dense routing):
    hidden = einsum("bnd,edh->benh", x, w1); hidden = gelu(hidden)
    out    = einsum("benh,ehd->bnde", hidden, w2)        # [b, n, d4, e]

Sharding: expert-parallel, 2 experts per core (16 experts / 8 cores); x is
replicated. Each core computes, for its experts e:
    hT[e] = gelu(W1[e].T @ X.T)        # [h, tok] layout, h on partitions
    outT[e] = W2[e].T @ hT[e]          # [d4, tok] layout
which keeps the contraction dim on SBUF partitions for both matmuls with no
on-device transposes. All matmul operands are bf16 (hidden is written back
from PSUM as bf16 by the gelu activation): the PE runs at 1 row/cycle with
FWL weight loads fully hidden — a back-to-back 216ns/matmul stream.  PSUM
accumulation stays fp32; output is stored bf16 and upcast on the host;
end-to-end rel err ~4e-3.

Inputs are host-packed so every input DMA reads 2-4KB contiguous per
partition (xp: [p, t, dt, tok]; w1/w2: [e, p, kt, free]) — small 1KB-line
patterns only reach ~180GB/s and starve the first iteration otherwise.  All
input DMAs ride the sync HWDGE ring in strict consumption order (a parallel
ring only steals HBM bandwidth from the critical path); completion lands
~2us after issue, so the critical first-expert data goes out as four 256KB
chunks.  The final store's PSUM->bf16 cast is split across vector+scalar to
shorten the tail.

The [e, d4, tok] device layout is re-interleaved to [b, n, d4, e] on the host.
"""

import sys

import numpy as np

for _p in ("/opt/trn_rl_repo", "/root/.axon_site/_ro/trn_rl_repo"):
    if _p not in sys.path:
        sys.path.append(_p)

import ml_dtypes

import concourse.bacc as bacc
import concourse.mybir as mybir
import concourse.tile as tile
from concourse.bass_utils import run_bass_kernel_spmd

F32 = mybir.dt.float32
BF16 = mybir.dt.bfloat16
NP_BF16 = ml_dtypes.bfloat16

N_CORES = 8
E = 16                 # total experts
E_LOC = E // N_CORES   # experts per core
D = 512                # model dim (contraction of mm1)
H = 512                # hidden dim (contraction of mm2)
D4 = 128               # output dim per expert
NTOK = 4 * 2048        # tokens
TT = 512               # token tile (matmul moving free dim)
P = 128
N_T = NTOK // TT
N_DT = D // P          # 4 k-tiles of mm1
N_HT = H // P          # 4 k-tiles of mm2


def _build_program():
    nc = bacc.Bacc("TRN2", target_bir_lowering=False, debug=False)
    # Host-packed layouts: partition dim explicit, DMA lines contiguous.
    xp = nc.declare_dram_parameter("xp", [P, N_T, N_DT, TT], BF16, isOutput=False)
    w1 = nc.declare_dram_parameter("w1", [E_LOC, P, N_DT, H], BF16, isOutput=False)
    w2 = nc.declare_dram_parameter("w2", [E_LOC, P, N_HT, D4], BF16, isOutput=False)
    outT = nc.declare_dram_parameter("outT", [E_LOC, D4, NTOK], BF16, isOutput=True)

    gelu = mybir.ActivationFunctionType.Gelu

    with tile.TileContext(nc) as tc:
        with (
            tc.tile_pool(name="wpool", bufs=1) as wpool,
            tc.tile_pool(name="xpool", bufs=4) as xpool,
            tc.tile_pool(name="hpool", bufs=2) as hpool,
            tc.tile_pool(name="opool", bufs=4) as opool,
            tc.tile_pool(name="ps1p", bufs=4, space="PSUM") as ps1p,
            tc.tile_pool(name="ps2p", bufs=4, space="PSUM") as ps2p,
        ):
            # Weights resident in SBUF for the whole kernel.
            w1_sb = wpool.tile([P, E_LOC, N_DT, H], BF16, name="w1_sb", tag="w1")
            w1_r = w1.rearrange("e p dt h -> p e dt h")
            w2_sb = wpool.tile([P, E_LOC, N_HT, D4], BF16, name="w2_sb", tag="w2")
            w2_r = w2.rearrange("e p ht d -> p e ht d")

            x_tiles = {}

            def load_x(t):
                x_sb = xpool.tile([P, N_DT, TT], BF16, name="x_sb", tag="x")
                nc.sync.dma_start(x_sb, xp[:, t])
                x_tiles[t] = x_sb

            # HAM warm-up: the PE starts throttled (K=4/8, ~1.2GHz) and only
            # un-throttles after ~3.4us of continuous execution.  The PE sits
            # idle waiting for the first DMAs anyway, so burn that window on
            # dummy matmuls over a zeroed scratch tile — real matmuls then
            # start at the warm 216ns cadence instead of 427ns.
            scratch = wpool.tile([P, TT], BF16, name="warm_sb", tag="warm")
            nc.gpsimd.memset(scratch, 0)
            warm_ps = [
                ps1p.tile([P, TT], F32, name=f"warm_ps{i}", tag="ps1")
                for i in range(2)
            ]
            for i in range(6):
                nc.tensor.matmul(warm_ps[i % 2], scratch[:, 0:P], scratch)

            # Startup in strict consumption order on sync: 256KB chunks.
            x0_sb = xpool.tile([P, N_DT, TT], BF16, name="x_sb", tag="x")
            nc.sync.dma_start(w1_sb[:, 0, 0:2], w1_r[:, 0, 0:2])
            nc.sync.dma_start(x0_sb[:, 0:2], xp[:, 0, 0:2])
            nc.sync.dma_start(w1_sb[:, 0, 2:4], w1_r[:, 0, 2:4])
            nc.sync.dma_start(x0_sb[:, 2:4], xp[:, 0, 2:4])
            x_tiles[0] = x0_sb
            for e in range(1, E_LOC):
                nc.sync.dma_start(w1_sb[:, e], w1_r[:, e])
            for e in range(E_LOC):
                nc.sync.dma_start(w2_sb[:, e], w2_r[:, e])

            for t in range(N_T):
                x_sb = x_tiles.pop(t)
                hT_tiles = []
                for e in range(E_LOC):
                    if e == 1 and t + 1 < N_T and t + 1 not in x_tiles:
                        load_x(t + 1)  # prefetch next tile, mid-iteration
                    hT_sb = hpool.tile([P, N_HT, TT], BF16, name="hT_sb", tag="h")
                    for ht in range(N_HT):
                        ps1 = ps1p.tile([P, TT], F32, name="ps1", tag="ps1")
                        for dt_i in range(N_DT):
                            nc.tensor.matmul(
                                ps1,
                                w1_sb[:, e, dt_i, ht * P : (ht + 1) * P],
                                x_sb[:, dt_i],
                                start=(dt_i == 0),
                                stop=(dt_i == N_DT - 1),
                            )
                        nc.scalar.activation(hT_sb[:, ht, :], ps1, gelu)
                    hT_tiles.append(hT_sb)
                tok = slice(t * TT, (t + 1) * TT)
                for e in range(E_LOC):
                    ps2 = ps2p.tile([P, TT], F32, name="ps2", tag="ps2")
                    for ht in range(N_HT):
                        nc.tensor.matmul(
                            ps2,
                            w2_sb[:, e, ht, :],
                            hT_tiles[e][:, ht, :],
                            start=(ht == 0),
                            stop=(ht == N_HT - 1),
                        )
                    o_sb = opool.tile([P, TT], BF16, name="o_sb", tag="o")
                    if t == N_T - 1 and e == E_LOC - 1:
                        # Final store is the critical tail: cast in two
                        # halves on vector+scalar concurrently.
                        nc.vector.tensor_copy(o_sb[:, : TT // 2], ps2[:, : TT // 2])
                        nc.scalar.activation(
                            o_sb[:, TT // 2 :],
                            ps2[:, TT // 2 :],
                            mybir.ActivationFunctionType.Copy,
                        )
                    else:
                        nc.vector.tensor_copy(o_sb, ps2)
                    nc.sync.dma_start(outT[e, :, tok], o_sb)

    nc.finalize()
    return nc


_NC = None


def _get_program():
    global _NC
    if _NC is None:
        _NC = _build_program()
    return _NC


def _in_maps(x: np.ndarray, w1: np.ndarray, w2: np.ndarray):
    # xp[p, t, dt, tok] = x.T[dt*128+p, t*512+tok]
    xT = x.reshape(NTOK, D).T.astype(NP_BF16)          # [D, NTOK]
    xp = np.ascontiguousarray(
        xT.reshape(N_DT, P, N_T, TT).transpose(1, 2, 0, 3)
    )
    # w1p[e, p, dt, h] = w1[e, dt*128+p, h]; same scheme for w2.
    w1b = w1.astype(NP_BF16).reshape(E, N_DT, P, H).transpose(0, 2, 1, 3)
    w2b = w2.astype(NP_BF16).reshape(E, N_HT, P, D4).transpose(0, 2, 1, 3)
    return [
        {
            "xp": xp,
            "w1": np.ascontiguousarray(w1b[c * E_LOC : (c + 1) * E_LOC]),
            "w2": np.ascontiguousarray(w2b[c * E_LOC : (c + 1) * E_LOC]),
        }
        for c in range(N_CORES)
    ]


def kernel(x: np.ndarray, w1: np.ndarray, w2: np.ndarray, **_) -> np.ndarray:
    """Full inputs in, full output out; expert-parallel across 8 NeuronCores."""
    nc = _get_program()
    res = run_bass_kernel_spmd(nc, _in_maps(x, w1, w2), list(range(N_CORES)))

    full = np.stack(
        [res.results[c]["outT"].astype(np.float32) for c in range(N_CORES)], axis=0
    )
    full = full.reshape(E, D4, NTOK)              # [e, d4, tok]
    out = full.transpose(2, 1, 0)                 # [tok, d4, e]
    return np.ascontiguousarray(out.reshape(4, 2048, D4, E), dtype=np.float32)
